# revision 50
# baseline (speedup 1.0000x reference)
"""Trainium2 Bass kernel for nn_ExpandOperator (banded scatter of a linear projection).

Reference semantics:
    pred = x @ W.T + b                      # (B, S, 2048)
    pred = pred.reshape(B, S, 64, 32)
    out[b, t, (t+s) % S, d] = pred[b, t, s, d]   # rest of out is zeros
    out shape: (B, S, S, 32) fp32  == 1 GiB

Sharding: 8 cores = (batch b in {0,1}) x (512-row seq chunk cc in {0..3}).
Each core computes pred for its 512 rows and owns its 128 MiB output slice.

Core-invariant SPMD trick: every core runs the identical program pretending its
rows are t = 0..511, so the scatter band sits on a fixed diagonal with no
wraparound.  The host rotates each core's block along the S axis by 512*cc
when unsharding (pure block memcpy).

Diagonal layout: the per-core output is declared as (512, 65568) where row t is
[2048-float band | 63520 floats of zeros].  Row-major linearization of this
buffer equals the true (512, 2048, 32) slice (band t lives at flat offset
65536*t + 32*t = 65568*t) plus a 64 KiB tail pad that the host drops.

Variant "band" (default): the kernel writes ONLY the band columns.  Both
run_bass_kernel_spmd execution paths hand the NEFF a pre-zeroed output buffer
(native: np.zeros -> run_neff; axon/PJRT: donated np.zeros with an explicit
"kernels that don't write every element rely on that" contract), so the gap
bytes are already zero and writing them again is pure waste: 124 of the 128
MiB of HBM store traffic disappears.  Matmul inputs are cast to bf16 on the
host (tolerance is 2e-2; bf16 error here is ~1e-3 of scale), the contraction
accumulates in fp32 PSUM, and the band is stored as fp32.

Per-core program (band):
  - wx dram [6, 128, 2560] bf16: k-tile k holds rows 128k..128k+127 of
    [W.T | x.T] (cols 0:2048 = W.T, 2048:2560 = x.T).  Loaded as 6 separate
    DMAs so the k=0 matmuls start after ~1/6 of the load.
  - for mb in 0..3 (128-row output blocks): 4 PSUM banks accumulate the four
    512-wide N-chunks across k; consecutive matmuls at the same (mb,k) share
    the stationary operand.  Bias (when nonzero) is a trailing K=1 matmul from
    a separate 1-row [b | ones] tensor.
  - DVE copies PSUM->SBUF; one 1 MiB band-store DMA per mb, alternating
    between the sync and gpsimd rings.

This walrus build only leaves room for ONE sync-wait per compute instruction;
_split_multi_waits rewrites any multi-wait Tile emits into single-wait NOP
chains on the same queue.
"""

import numpy as np
import ml_dtypes

import bass_rust
import concourse.bass as bass
import concourse.mybir as mybir
import concourse.tile as tile
from concourse.bass_utils import run_bass_kernel_spmd

F32 = mybir.dt.float32
BF16 = mybir.dt.bfloat16
FP8 = mybir.dt.float8e4
NP_BF16 = ml_dtypes.bfloat16
NP_FP8 = mybir.dt.np(FP8)


def _split_multi_waits(nc):
    """Walrus in this toolchain only leaves ONE sync-wait slot per
    instruction.  Tile's tail drain waits on every semaphore lane it used,
    which fails codegen.  Hoist all-but-one wait of any multi-wait
    instruction into single-wait NOPs on the same engine queue immediately
    before it - semantically identical (same-queue waits execute in order).
    """
    eng_by_type = {
        mybir.EngineType.SP: nc.sync,
        mybir.EngineType.PE: nc.tensor,
        mybir.EngineType.Activation: nc.scalar,
        mybir.EngineType.Pool: nc.gpsimd,
        mybir.EngineType.DVE: nc.vector,
    }
    tail_bb = nc.cur_bb.bb
    for f in nc.m.functions:
        for bb in f.blocks:
            il = bb.instructions
            i = 0
            while i < len(il):
                ins = il[i]
                si = getattr(ins, "sync_info", None)
                if si is not None and len(si.on_wait) > 1:
                    waits = list(si.on_wait)
                    for w in waits[:-1]:
                        nop = eng_by_type[ins.engine].nop(nofuse=True).ins
                        tail_bb.instructions.remove(nop)
                        nop.sync_info = bass_rust.SyncInfo(
                            on_wait=[w], on_update=[])
                        il.insert(i, nop)
                        i += 1
                    ins.sync_info = bass_rust.SyncInfo(
                        on_wait=[waits[-1]], on_update=list(si.on_update))
                i += 1

def _dedupe_ldweights(nc):
    """TRN2's PE does not hide the stationary reload: each self-loading
    matmul legalizes to Ldweights+Matmult and the ~53ns load serializes with
    the 213ns stream.  The weight registers persist across matmuls, so a
    run of matmuls sharing the same stationary AP only needs the first
    Ldweights.  Drop the redundant ones (only those carrying no syncs)."""
    for f in nc.m.functions:
        for bb in f.blocks:
            il = bb.instructions
            last = None
            i = 0
            while i < len(il):
                ins = il[i]
                if ins.engine == mybir.EngineType.PE:
                    if ins.opcode == "Ldweights":
                        sig = (repr(ins.ins[0]), ins.perf_mode,
                               ins.is_transpose, ins.tile_position,
                               ins.tile_size)
                        si = getattr(ins, "sync_info", None)
                        clean = si is None or (
                            not si.on_wait and not si.on_update)
                        if last == sig and clean:
                            del il[i]
                            continue
                        last = sig
                    elif ins.opcode != "Matmult":
                        last = None
                i += 1


# Problem shapes (hardcoded per contract).
B = 2
S = 2048
D_IN = 768
MAX_SPAN = 64
SPAN_DIM = 32
N_OUT = MAX_SPAN * SPAN_DIM  # 2048
N_CORES = 8
CHUNKS = 4                   # seq chunks per batch (B * CHUNKS == N_CORES)
ROWS = S // CHUNKS           # 512 rows per core

VARIANT = "band"             # "band" | "v1" (full-write fp32 fallback)
IN_DT = "bf16"               # "bf16" | "fp8" (DoubleRow) matmul input dtype
OUT_DT = "bf16"              # "f32" | "bf16" band store dtype (host upcasts)
TIMING_REPEATS = (24, 72)    # repeat-differencing points for timing.py


def build_nc(repeats=1, variant=None, with_bias=False, in_dt=None,
             out_dt=None, **kw):
    v = VARIANT if variant is None else variant
    if v == "band":
        return build_nc_band(repeats=repeats, with_bias=with_bias,
                             in_dt=IN_DT if in_dt is None else in_dt,
                             out_dt=OUT_DT if out_dt is None else out_dt,
                             **kw)
    return build_nc_v1(repeats=repeats)


def build_nc_band(rows=ROWS, s=S, d_in=D_IN, n_out=N_OUT, span_dim=SPAN_DIM,
                  repeats=1, with_bias=False, in_dt="bf16", out_dt="f32",
                  skip_store=False, skip_mm=False, skip_load=False,
                  n_loads=6, k_levels=None, wx_bufs=2, dedupe_ldw=True,
                  store_split=1, load_plan="fast", last_store_split=2,
                  copy_split=True, warmup=24, psum_store=False,
                  first_chunks=2):
    """Band-only variant: write just the diagonal band, bf16/fp8 matmul."""
    period = s * span_dim + span_dim     # 65568
    kt = d_in // 128                     # 6
    mblk = rows // 128                   # 4
    nw = 512                             # one fp32 PSUM bank
    nchunk = n_out // nw                 # 4
    wcols = n_out + rows                 # 2560
    idt = {"bf16": BF16, "fp8": FP8}[in_dt]
    odt = {"f32": F32, "bf16": BF16}[out_dt]
    # fp8 DoubleRow: one matmul consumes two 128-row K-tiles packed along
    # the free dim (lhsT free 256 -> out partition 128, rhs free 1024 ->
    # out free 512), streaming 2 fp8 rows per cycle.
    kstep = 2 if in_dt == "fp8" else 1
    dr = mybir.MatmulPerfMode.DoubleRow if in_dt == "fp8" else None

    nc = bass.Bass()
    wx = nc.dram_tensor("wx", [kt, 128, wcols], idt, kind="ExternalInput")
    wx_r = wx.rearrange("k p m -> p k m")            # (128, kt, wcols)
    if with_bias:
        aux = nc.dram_tensor("aux", [1, wcols], idt, kind="ExternalInput")
    out = nc.dram_tensor("out", [rows, period], odt, kind="ExternalOutput")

    kl = kt if k_levels is None else k_levels

    with tile.TileContext(nc) as tc:
        with (
            tc.tile_pool(name="const", bufs=1) as cpool,
            tc.tile_pool(name="wxp", bufs=wx_bufs) as wxpool,
            tc.tile_pool(name="pred", bufs=2) as ppool,
            tc.tile_pool(name="psum", bufs=2, space="PSUM") as pspool,
        ):
            if skip_load:
                wx_init = wxpool.tile([128, kt, wcols], idt, tag="wx_sb")
                nc.vector.memset(wx_init[:], 0.0)
            if warmup:
                # PE p-state ramps to max only after ~3us of continuous
                # execution; run tiny matmuls on junk data while the first
                # loads are in flight so the real stream starts at full
                # clock.  They write a PSUM bank the first real start=True
                # matmul resets, so results are unaffected.
                wu = cpool.tile([128, 192], idt, tag="wu")
                nc.vector.memset(wu[:], 0.0)
                psw = pspool.tile([128, 64], F32, tag="ps0", name="psw")
                for _ in range(warmup):
                    nc.tensor.matmul(psw[:], wu[:, 0:128], wu[:, 128:192],
                                     start=True, stop=True)
            for _rep in range(repeats):
                if skip_load:
                    wx_sb = wx_init
                elif load_plan in ("fast", "fast2"):
                    # Layout is [x | W], so one contiguous piece covers the
                    # first matmuls' operands (x + W n0[/n1]) and lands ~2us
                    # earlier than a full k-tile; fast2 spreads the odd
                    # k-tiles onto the gpsimd (SWDGE) ring.
                    cut = rows + first_chunks * nw
                    wx_sb = wxpool.tile([128, kt, wcols], idt, tag="wx_sb")
                    nc.scalar.dma_start(wx_sb[:, 0, 0:cut],
                                        wx_r[:, 0, 0:cut])
                    nc.scalar.dma_start(wx_sb[:, 0, cut:],
                                        wx_r[:, 0, cut:])
                    for k in range(1, kt):
                        eng = (nc.gpsimd
                               if load_plan == "fast2" and k % 2 == 1
                               else nc.scalar)
                        eng.dma_start(wx_sb[:, k, :], wx_r[:, k, :])
                else:
                    wx_sb = wxpool.tile([128, kt, wcols], idt, tag="wx_sb")
                    kper = kt // n_loads
                    for k0 in range(0, kt, kper):
                        nc.scalar.dma_start(wx_sb[:, k0:k0 + kper, :],
                                            wx_r[:, k0:k0 + kper, :])
                if with_bias:
                    bias_sb = cpool.tile([1, wcols], idt, tag="bias_sb")
                    nc.scalar.dma_start(bias_sb[:], aux[0:1, :])

                for mb in range(mblk):
                    rs = mb * 128
                    store_engs = ([nc.sync] if load_plan == "fast2"
                                  else [nc.sync, nc.gpsimd])
                    if not psum_store:
                        pt = ppool.tile([128, n_out], odt)
                    pss = [pspool.tile([128, nw], F32, tag=f"ps{n}",
                                       name=f"ps{n}")
                           for n in range(nchunk)]
                    if skip_mm:
                        continue
                    for k in range(0, kl, kstep):
                        for n in range(nchunk):
                            nc.tensor.matmul(
                                pss[n][:],
                                wx_sb[:, k:k + kstep, rs:rs + 128],
                                wx_sb[:, k:k + kstep,
                                      rows + n * nw:rows + (n + 1) * nw],
                                start=(k == 0),
                                stop=(k + kstep >= kl and not with_bias),
                                perf_mode=dr,
                            )
                    if with_bias:
                        for n in range(nchunk):
                            nc.tensor.matmul(
                                pss[n][:],
                                bias_sb[:, rs:rs + 128],
                                bias_sb[:, rows + n * nw:rows + (n + 1) * nw],
                                start=False,
                                stop=True,
                            )
                    if skip_store:
                        continue
                    if psum_store:
                        # fp32 band stored straight from the PSUM banks as
                        # each accumulation group stops - no copy stage, no
                        # copy tail after the last matmul.
                        for n in range(nchunk):
                            eng = store_engs[(mb * nchunk + n)
                                             % len(store_engs)]
                            eng.dma_start(
                                out[rs:rs + 128, n * nw:(n + 1) * nw],
                                pss[n][:])
                        continue
                    if copy_split and mb == mblk - 1:
                        # last block's copies are the serial tail after the
                        # final matmuls - alternate the two PSUM-capable
                        # copy engines (GPSIMD cannot read PSUM)
                        cengs = [nc.vector.tensor_copy, nc.scalar.copy,
                                 nc.vector.tensor_copy, nc.scalar.copy]
                        for n in range(nchunk):
                            cengs[n](pt[:, n * nw:(n + 1) * nw], pss[n][:])
                    else:
                        for n in range(nchunk):
                            nc.vector.tensor_copy(pt[:, n * nw:(n + 1) * nw],
                                                  pss[n][:])
                    ssp = store_split
                    if mb == mblk - 1:
                        ssp = max(ssp, last_store_split)
                    if ssp == 1:
                        eng = store_engs[mb % len(store_engs)]
                        eng.dma_start(out[rs:rs + 128, 0:n_out], pt[:])
                    else:
                        sw = n_out // ssp
                        for g in range(ssp):
                            eng = store_engs[(mb * ssp + g) % len(store_engs)]
                            eng.dma_start(
                                out[rs:rs + 128, g * sw:(g + 1) * sw],
                                pt[:, g * sw:(g + 1) * sw])

    if dedupe_ldw:
        _dedupe_ldweights(nc)
    _split_multi_waits(nc)
    return nc


def build_nc_v1(rows=ROWS, s=S, d_in=D_IN, n_out=N_OUT, span_dim=SPAN_DIM,
                gap_split=8, repeats=1):
    """Full-write fp32 fallback (writes gap zeros itself; ~420us/core).

    Inputs (per core):
      wx : (d_pad, n_out + rows)  [Waug.T | x_aug.T] packed -> one DMA load,
           d_pad = round_up(d_in + 1, 128); row d_in = [b | 1.0s], rest 0.
    Output:
      out: (rows, period) diagonal-layout buffer, period = s*span_dim + span_dim
    """
    row_f = s * span_dim            # true floats per output row
    period = row_f + span_dim       # diagonal period (band marches span_dim/row)
    gap = period - n_out            # zero floats after each band
    assert gap % gap_split == 0
    gw = gap // gap_split           # floats per gap-chunk DMA
    d_pad = -(-(d_in + 1) // 128) * 128
    kt = d_pad // 128               # contraction tiles (incl. bias tile)
    mblk = rows // 128              # 128-row blocks
    nw = min(512, n_out)            # psum chunk width (one fp32 bank)
    nchunk = n_out // nw
    wcols = n_out + rows            # packed free width

    nc = bass.Bass()
    wx = nc.dram_tensor("wx", [d_pad, wcols], F32, kind="ExternalInput")
    out = nc.dram_tensor("out", [rows, period], F32, kind="ExternalOutput")

    wx_r = wx.rearrange("(k p) m -> p k m", p=128)   # (128, kt, wcols)

    with tile.TileContext(nc) as tc:
        with (
            tc.tile_pool(name="const", bufs=1) as cpool,
            tc.tile_pool(name="pred", bufs=mblk) as ppool,
            tc.tile_pool(name="psum", bufs=4, space="PSUM") as pspool,
        ):
            # Zero source tile for the gap writes.
            zt = cpool.tile([128, gw], F32)
            nc.vector.memset(zt[:], 0.0)

            for _rep in range(repeats):
                # Gap writes: everything after each band, uniform strided
                # DMAs.  These only depend on the memset, so they start
                # immediately.
                for mb in range(mblk):
                    rs = mb * 128
                    for g in range(gap_split):
                        cs = n_out + g * gw
                        nc.sync.dma_start(out[rs:rs + 128, cs:cs + gw], zt[:])

                # Weights + activations + bias row in one DMA.
                wx_sb = cpool.tile([128, kt, wcols], F32, tag="wx_sb")
                nc.scalar.dma_start(wx_sb[:], wx_r[:])

                # pred = x @ W.T + b, one 128-row block at a time.
                for mb in range(mblk):
                    rs = mb * 128
                    pt = ppool.tile([128, n_out], F32)
                    for n in range(nchunk):
                        ns = n * nw
                        ps = pspool.tile([128, nw], F32)
                        for k in range(kt):
                            nc.tensor.matmul(
                                ps[:],
                                wx_sb[:, k, n_out + rs:n_out + rs + 128],
                                wx_sb[:, k, ns:ns + nw],
                                start=(k == 0),
                                stop=(k == kt - 1),
                            )
                        nc.vector.tensor_copy(pt[:, ns:ns + nw], ps[:])
                    nc.gpsimd.dma_start(out[rs:rs + 128, 0:n_out], pt[:])

    _split_multi_waits(nc)
    return nc


_CACHE = {}


def _get_nc(variant, with_bias):
    key = (variant, with_bias, IN_DT, OUT_DT)
    if key not in _CACHE:
        _CACHE[key] = build_nc(variant=variant, with_bias=with_bias)
    return _CACHE[key]


def make_in_maps(x, W, b, variant=None, with_bias=False, in_dt=None):
    """Host-side sharding: per-core packed input dicts."""
    v = VARIANT if variant is None else variant
    x = np.asarray(x, np.float32)
    W = np.asarray(W, np.float32)
    b = np.asarray(b, np.float32)
    in_maps = []
    if v == "band":
        np_idt = {"bf16": NP_BF16, "fp8": NP_FP8}[
            IN_DT if in_dt is None else in_dt]
        kt = D_IN // 128
        wcols = N_OUT + ROWS
        Wt = W.T.astype(np_idt)                       # (768, 2048)
        if with_bias:
            aux_np = np.empty((1, wcols), np_idt)
            aux_np[0, :ROWS] = np_idt(1.0)
            aux_np[0, ROWS:] = b.astype(np_idt)
        for c in range(N_CORES):
            bi, cc = divmod(c, CHUNKS)
            xs = x[bi, cc * ROWS:(cc + 1) * ROWS, :]
            wx_np = np.empty((D_IN, wcols), np_idt)
            wx_np[:, :ROWS] = xs.T.astype(np_idt)
            wx_np[:, ROWS:] = Wt
            m = {"wx": wx_np.reshape(kt, 128, wcols)}
            if with_bias:
                m["aux"] = aux_np
            in_maps.append(m)
        return in_maps

    d_pad = -(-(D_IN + 1) // 128) * 128  # 896
    for c in range(N_CORES):
        bi, cc = divmod(c, CHUNKS)
        xs = x[bi, cc * ROWS:(cc + 1) * ROWS, :]
        wx_np = np.zeros((d_pad, N_OUT + ROWS), np.float32)
        wx_np[:D_IN, :N_OUT] = W.T
        wx_np[:D_IN, N_OUT:] = xs.T
        wx_np[D_IN, :N_OUT] = b
        wx_np[D_IN, N_OUT:] = 1.0
        in_maps.append({"wx": wx_np})
    return in_maps


def unshard(results):
    """Host-side unsharding: drop tail pad, rotate along S by 512*cc, place."""
    row_f = S * SPAN_DIM
    out = np.empty((B, S, S, SPAN_DIM), np.float32)
    for c in range(N_CORES):
        bi, cc = divmod(c, CHUNKS)
        buf = np.asarray(results[c]["out"])
        local = buf.reshape(-1)[:ROWS * row_f].reshape(ROWS, S, SPAN_DIM)
        sh = cc * ROWS
        blk = out[bi, sh:sh + ROWS]
        if sh:
            blk[:, sh:, :] = local[:, :S - sh, :]
            blk[:, :sh, :] = local[:, S - sh:, :]
        else:
            blk[:, :, :] = local
    return out


def kernel(x, W, b):
    x = np.asarray(x)
    W = np.asarray(W)
    b = np.asarray(b)
    with_bias = VARIANT == "band" and bool(np.any(b != 0))
    nc = _get_nc(VARIANT, with_bias)
    res = run_bass_kernel_spmd(nc, make_in_maps(x, W, b, with_bias=with_bias),
                               list(range(N_CORES)))
    return unshard(res.results)
